# revision 5
# baseline (speedup 1.0000x reference)
import time

import numpy as np
import ml_dtypes

from concourse import bacc, mybir, tile, bass_utils
from concourse.masks import make_identity

B, S, V, D, H, DH, L = 2, 1024, 50257, 512, 8, 64, 4
DM = 2048
T = B * S
TLOC = 256
VPAD = 51200
VLOC = VPAD // 8
EPS = 1e-5
NCORES = 8
GROUPS_KV = [[0, 1, 2, 3], [4, 5, 6, 7]]
GROUP_ALL = [list(range(NCORES))]
KFLAT = D * TLOC  # 131072, also == TLOC * D for the v slot

FP32 = mybir.dt.float32
BF16 = mybir.dt.bfloat16
AF = mybir.ActivationFunctionType

_CACHE = {}
last_run_s = None


def _build():
    nc = bacc.Bacc("TRN2", target_bir_lowering=False, debug=False, num_devices=NCORES)
    h0_e = nc.dram_tensor("h0", [TLOC, D], FP32, kind="ExternalInput").ap()
    mask_e = nc.dram_tensor("mask", [S, TLOC], BF16, kind="ExternalInput").ap()
    wq_e = nc.dram_tensor("wq", [L, D, D], BF16, kind="ExternalInput").ap()
    wk_e = nc.dram_tensor("wk", [L, D, D], BF16, kind="ExternalInput").ap()
    wv_e = nc.dram_tensor("wv", [L, D, D], BF16, kind="ExternalInput").ap()
    wo_e = nc.dram_tensor("wo", [L, D, D], BF16, kind="ExternalInput").ap()
    wi_e = nc.dram_tensor("wi", [L, D, DM], BF16, kind="ExternalInput").ap()
    wu_e = nc.dram_tensor("wout", [L, DM, D], BF16, kind="ExternalInput").ap()
    gamma_e = nc.dram_tensor("gamma", [L, D], FP32, kind="ExternalInput").ap()
    beta_e = nc.dram_tensor("beta", [L, D], FP32, kind="ExternalInput").ap()
    emb_e = nc.dram_tensor("embT", [D, VLOC], BF16, kind="ExternalInput").ap()
    out_e = nc.dram_tensor("out", [T, VLOC], FP32, kind="ExternalOutput").ap()

    with tile.TileContext(nc) as tc:
        with (
            tc.tile_pool(name="const", bufs=1) as const,
            tc.tile_pool(name="state", bufs=1) as state,
            tc.tile_pool(name="wts", bufs=1) as wts,
            tc.tile_pool(name="act", bufs=1) as act,
            tc.tile_pool(name="pipe", bufs=3) as pipe,
            tc.tile_pool(name="ps", bufs=2, space="PSUM") as ps,
            tc.tile_pool(name="dram", bufs=2, space="DRAM") as dram,
        ):
            ident = const.tile([128, 128], FP32, name="ident")
            make_identity(nc, ident)
            eps_sb = const.tile([128, 1], FP32, name="eps_sb")
            nc.vector.memset(eps_sb[:], EPS)
            mask_sb = const.tile([128, 8, TLOC], BF16, name="mask_sb")
            nc.sync.dma_start(
                mask_sb[:], mask_e.rearrange("(kt ki) q -> ki kt q", ki=128)
            )
            emb_sb = const.tile([128, 4, VLOC], BF16, name="emb_sb")
            nc.sync.dma_start(
                emb_sb[:], emb_e.rearrange("(ko ki) v -> ki ko v", ki=128)
            )

            h_sb = state.tile([128, 2, D], FP32, name="h_sb")
            nc.sync.dma_start(
                h_sb[:], h0_e.rearrange("(t ti) d -> ti t d", ti=128)
            )

            # persistent activations
            hT = act.tile([128, 4, TLOC], BF16, name="hT")
            qT = act.tile([128, 4, TLOC], BF16, name="qT")
            kT = act.tile([128, 4, TLOC], BF16, name="kT")
            zT = act.tile([128, 4, TLOC], BF16, name="zT")
            vloc = act.tile([128, 2, D], BF16, name="vloc")
            kTg = act.tile([128, 4, 4, TLOC], BF16, name="kTg")
            vg = act.tile([128, 8, H, DH + 1], BF16, name="vg")
            uT = act.tile([128, 16, TLOC], BF16, name="uT")
            htg = act.tile([128, 4, NCORES, TLOC], BF16, name="htg")
            for tt in range(8):
                nc.vector.memset(vg[:, tt, :, DH:DH + 1], 1.0)

            def transpose_h(dst):
                for t in range(2):
                    for f in range(4):
                        tp = ps.tile([128, D], FP32, name="mm")
                        nc.tensor.transpose(
                            tp[:, 0:128], h_sb[:, t, 128 * f:128 * f + 128], ident[:]
                        )
                        nc.vector.tensor_copy(
                            dst[:, f, 128 * t:128 * t + 128], tp[:, 0:128]
                        )

            for l in range(L):
                # ---- per-layer weights ----
                wq_sb = wts.tile([128, 4, D], BF16, name="wq_sb")
                wk_sb = wts.tile([128, 4, D], BF16, name="wk_sb")
                wv_sb = wts.tile([128, 4, D], BF16, name="wv_sb")
                wo_sb = wts.tile([128, 4, D], BF16, name="wo_sb")
                wi_sb = wts.tile([128, 4, DM], BF16, name="wi_sb")
                wu_sb = wts.tile([128, 16, D], BF16, name="wu_sb")
                for dst, src in (
                    (wq_sb, wq_e), (wk_sb, wk_e), (wv_sb, wv_e), (wo_sb, wo_e)
                ):
                    nc.sync.dma_start(
                        dst[:], src[l].rearrange("(ko ki) m -> ki ko m", ki=128)
                    )
                nc.sync.dma_start(
                    wi_sb[:], wi_e[l].rearrange("(ko ki) m -> ki ko m", ki=128)
                )
                nc.sync.dma_start(
                    wu_sb[:], wu_e[l].rearrange("(ko ki) m -> ki ko m", ki=128)
                )
                g1 = wts.tile([1, D], FP32, name="g1")
                b1 = wts.tile([1, D], FP32, name="b1")
                gb = wts.tile([128, D], FP32, name="gb")
                bb = wts.tile([128, D], FP32, name="bb")
                nc.sync.dma_start(g1[:], gamma_e[l:l + 1, :])
                nc.sync.dma_start(b1[:], beta_e[l:l + 1, :])
                nc.gpsimd.partition_broadcast(gb[:], g1[:])
                nc.gpsimd.partition_broadcast(bb[:], b1[:])

                # ---- h -> hT (feature-major bf16) ----
                transpose_h(hT)

                # ---- q/k projections (feature-major out) ----
                for mo in range(4):
                    pq = ps.tile([128, TLOC], FP32, name="sc")
                    for kf in range(4):
                        nc.tensor.matmul(
                            pq[:], wq_sb[:, kf, 128 * mo:128 * mo + 128],
                            hT[:, kf, :], start=(kf == 0), stop=(kf == 3),
                        )
                    nc.scalar.activation(qT[:, mo, :], pq[:], AF.Copy, scale=0.125)
                for mo in range(4):
                    pk = ps.tile([128, TLOC], FP32, name="sc")
                    for kf in range(4):
                        nc.tensor.matmul(
                            pk[:], wk_sb[:, kf, 128 * mo:128 * mo + 128],
                            hT[:, kf, :], start=(kf == 0), stop=(kf == 3),
                        )
                    nc.vector.tensor_copy(kT[:, mo, :], pk[:])
                # ---- v projection (token-major out) ----
                for t in range(2):
                    pv = ps.tile([128, D], FP32, name="mm")
                    for kf in range(4):
                        nc.tensor.matmul(
                            pv[:], hT[:, kf, 128 * t:128 * t + 128],
                            wv_sb[:, kf, :], start=(kf == 0), stop=(kf == 3),
                        )
                    nc.scalar.activation(vloc[:, t, :], pv[:], AF.Copy)

                # ---- bounce k/v to DRAM, allgather within group of 4 ----
                kv_in = dram.tile([2, KFLAT], BF16, name="kv_in")
                kv_out = dram.tile([4, 2, KFLAT], BF16, name="kv_out")
                nc.gpsimd.dma_start(
                    kv_in.rearrange("c (f ki t) -> c ki f t", f=4, ki=128)[0], kT[:]
                )
                nc.gpsimd.dma_start(
                    kv_in.rearrange("c (tb ti d) -> c ti tb d", tb=2, ti=128)[1],
                    vloc[:],
                )
                nc.gpsimd.collective_compute(
                    "AllGather",
                    mybir.AluOpType.bypass,
                    replica_groups=GROUPS_KV,
                    ins=[kv_in.opt()],
                    outs=[kv_out.opt()],
                )
                ksrc = kv_out.rearrange("r c (f ki t) -> c ki f r t", f=4, ki=128)
                for r in range(4):
                    nc.sync.dma_start(kTg[:, :, r, :], ksrc[0, :, :, r, :])
                vsrc = kv_out.rearrange(
                    "r c (tb ti h e) -> c r tb ti h e", tb=2, ti=128, h=H
                )
                for tt in range(8):
                    nc.sync.dma_start(
                        vg[:, tt, :, 0:DH], vsrc[1, tt // 2, tt % 2]
                    )

                # ---- attention per head ----
                for hh in range(H):
                    fh = hh // 2
                    ph = 64 * (hh % 2)
                    zaug = ps.tile([DH + 1, TLOC], FP32, name="zaug")
                    for kt in range(8):
                        sc = ps.tile([128, TLOC], FP32, name="sc")
                        nc.tensor.matmul(
                            sc[:],
                            kTg[ph:ph + 64, fh, kt // 2,
                                128 * (kt % 2):128 * (kt % 2) + 128],
                            qT[ph:ph + 64, fh, :],
                            start=True, stop=True,
                        )
                        e = pipe.tile([128, TLOC], BF16, name="expS")
                        nc.scalar.activation(e[:], sc[:], AF.Exp)
                        nc.vector.tensor_mul(e[:], e[:], mask_sb[:, kt, :])
                        nc.tensor.matmul(
                            zaug[:], vg[:, kt, hh, :], e[:],
                            start=(kt == 0), stop=(kt == 7),
                        )
                    dn = pipe.tile([1, TLOC], FP32, name="dn")
                    rc = pipe.tile([1, TLOC], FP32, name="rc")
                    bcast = pipe.tile([128, TLOC], FP32, name="bcast")
                    nc.vector.tensor_copy(dn[:], zaug[DH:DH + 1, :])
                    nc.vector.reciprocal(rc[:], dn[:])
                    nc.gpsimd.partition_broadcast(bcast[:], rc[:])
                    nc.vector.tensor_mul(
                        zT[ph:ph + 64, fh, :], zaug[0:DH, :], bcast[0:64, :]
                    )

                # ---- output projection + residual ----
                for t in range(2):
                    po = ps.tile([128, D], FP32, name="mm")
                    for kf in range(4):
                        nc.tensor.matmul(
                            po[:], zT[:, kf, 128 * t:128 * t + 128],
                            wo_sb[:, kf, :], start=(kf == 0), stop=(kf == 3),
                        )
                    nc.vector.tensor_add(h_sb[:, t, :], h_sb[:, t, :], po[:])

                # ---- layernorm (token-major, free-axis reduce) ----
                for t in range(2):
                    sstat = pipe.tile([128, 1], FP32, name="sstat")
                    mu = pipe.tile([128, 1], FP32, name="mu")
                    ss = pipe.tile([128, 1], FP32, name="ss")
                    srt = pipe.tile([128, 1], FP32, name="srt")
                    ri = pipe.tile([128, 1], FP32, name="ri")
                    hc = pipe.tile([128, D], FP32, name="hc")
                    sq = pipe.tile([128, D], BF16, name="sq")
                    nc.vector.reduce_sum(
                        sstat[:], h_sb[:, t, :], axis=mybir.AxisListType.X
                    )
                    nc.vector.tensor_scalar_mul(mu[:], sstat[:], 1.0 / D)
                    nc.vector.tensor_scalar_sub(hc[:], h_sb[:, t, :], mu[:])
                    nc.scalar.activation(sq[:], hc[:], AF.Square, accum_out=ss[:])
                    nc.scalar.activation(
                        srt[:], ss[:], AF.Sqrt, bias=eps_sb[:], scale=1.0 / D
                    )
                    nc.vector.reciprocal(ri[:], srt[:])
                    nc.vector.tensor_scalar_mul(hc[:], hc[:], ri[:])
                    nc.vector.tensor_mul(hc[:], hc[:], gb[:])
                    nc.vector.tensor_add(h_sb[:, t, :], hc[:], bb[:])

                # ---- h -> hT again for MLP ----
                transpose_h(hT)

                # ---- MLP up + gelu (feature-major out) ----
                for mo in range(16):
                    pu = ps.tile([128, TLOC], FP32, name="sc")
                    for kf in range(4):
                        nc.tensor.matmul(
                            pu[:], wi_sb[:, kf, 128 * mo:128 * mo + 128],
                            hT[:, kf, :], start=(kf == 0), stop=(kf == 3),
                        )
                    nc.scalar.activation(uT[:, mo, :], pu[:], AF.Gelu)
                # ---- MLP down + residual ----
                for t in range(2):
                    pd = ps.tile([128, D], FP32, name="mm")
                    for km in range(16):
                        nc.tensor.matmul(
                            pd[:], uT[:, km, 128 * t:128 * t + 128],
                            wu_sb[:, km, :], start=(km == 0), stop=(km == 15),
                        )
                    nc.vector.tensor_add(h_sb[:, t, :], h_sb[:, t, :], pd[:])

            # ---- global hT allgather across all 8 cores ----
            transpose_h(hT)
            hg_in = dram.tile([KFLAT], BF16, name="hg_in")
            hg_out = dram.tile([NCORES, KFLAT], BF16, name="hg_out")
            nc.gpsimd.dma_start(
                hg_in.rearrange("(f ki t) -> ki f t", f=4, ki=128), hT[:]
            )
            nc.gpsimd.collective_compute(
                "AllGather",
                mybir.AluOpType.bypass,
                replica_groups=GROUP_ALL,
                ins=[hg_in.opt()],
                outs=[hg_out.opt()],
            )
            hsrc = hg_out.rearrange("r (f ki t) -> ki f r t", f=4, ki=128)
            for r in range(NCORES):
                nc.sync.dma_start(htg[:, :, r, :], hsrc[:, :, r, :])

            # ---- unembed: logits[128m:+128, col:col+nw] ----
            nvs = [512] * 12 + [256]
            for m in range(16):
                r = m // 2
                th = 128 * (m % 2)
                col = 0
                for nw in nvs:
                    up = ps.tile([128, D], FP32, name="mm")
                    for kf in range(4):
                        nc.tensor.matmul(
                            up[:, 0:nw], htg[:, kf, r, th:th + 128],
                            emb_sb[:, kf, col:col + nw],
                            start=(kf == 0), stop=(kf == 3),
                        )
                    ot = pipe.tile([128, D], FP32, name="ot")
                    nc.vector.tensor_copy(ot[:, 0:nw], up[:, 0:nw])
                    nc.sync.dma_start(
                        out_e[128 * m:128 * m + 128, col:col + nw], ot[:, 0:nw]
                    )
                    col += nw
    nc.compile()
    return nc


def _get_nc():
    if "nc" not in _CACHE:
        _CACHE["nc"] = _build()
    return _CACHE["nc"]


def kernel(**inputs):
    global last_run_s
    bf16 = ml_dtypes.bfloat16
    x = np.asarray(inputs["x"])
    embed = np.asarray(inputs["embed"], np.float32)
    pos = np.asarray(inputs["pos_embed"], np.float32)

    h0 = (embed[x.reshape(-1)] + np.tile(pos, (B, 1))).astype(np.float32)
    wq = np.ascontiguousarray(np.asarray(inputs["Q"], np.float32)).astype(bf16)
    wk = np.ascontiguousarray(np.asarray(inputs["K"], np.float32)).astype(bf16)
    wv = np.ascontiguousarray(
        np.transpose(np.asarray(inputs["Vw"], np.float32), (0, 2, 1, 3)).reshape(
            L, D, H * DH
        )
    ).astype(bf16)
    wo = np.ascontiguousarray(np.asarray(inputs["O"], np.float32)).astype(bf16)
    wi = np.ascontiguousarray(np.asarray(inputs["W_in"], np.float32)).astype(bf16)
    wu = np.ascontiguousarray(np.asarray(inputs["W_out"], np.float32)).astype(bf16)
    gamma = np.ascontiguousarray(np.asarray(inputs["gamma"], np.float32))
    beta = np.ascontiguousarray(np.asarray(inputs["beta"], np.float32))

    embP = np.zeros((VPAD, D), np.float32)
    embP[:V] = embed
    ks = np.arange(S)[:, None]
    in_maps = []
    for c in range(NCORES):
        qs = np.arange(TLOC)[None, :] + (c % 4) * TLOC
        mask = (ks <= qs).astype(np.float32).astype(bf16)
        embT = np.ascontiguousarray(
            embP[c * VLOC:(c + 1) * VLOC].T
        ).astype(bf16)
        in_maps.append({
            "h0": np.ascontiguousarray(h0[c * TLOC:(c + 1) * TLOC]),
            "mask": np.ascontiguousarray(mask),
            "embT": embT,
            "wq": wq, "wk": wk, "wv": wv, "wo": wo,
            "wi": wi, "wout": wu,
            "gamma": gamma, "beta": beta,
        })

    nc = _get_nc()
    t0 = time.perf_counter()
    res = bass_utils.run_bass_kernel_spmd(
        nc, in_maps, core_ids=list(range(NCORES)), trace=False
    )
    last_run_s = time.perf_counter() - t0

    logits = np.concatenate(
        [np.asarray(res.results[c]["out"]) for c in range(NCORES)], axis=1
    )
    return np.ascontiguousarray(
        logits[:, :V].reshape(B, S, V).astype(np.float32)
    )
